# revision 2
# baseline (speedup 1.0000x reference)
"""DimGraphConv (GNN mean-aggregation message passing) on 8 Trainium2 cores.

Math (per reference):
    out = x @ W_self.T + segment_mean(x[row], col) @ W_neigh.T + bias

v3 design: target-sharded (core c owns targets [c*12500, (c+1)*12500)); the
per-edge aggregation runs on the TENSOR engine instead of the CCE scatter-add:

  * Edges are bucketed by (128-target window w, gather class k).  A class is
    (source bank, row parity): x is stored as a bf16 row-pair array
    xpair[50000, 128] so one 256B gather descriptor fetches the row pair that
    contains the wanted source row; parity picks the 64-element half.
  * Each bucket has a fixed 256-slot capacity (2 matmul tiles).  Overflow
    (~2.5% of edges) is routed to a small CCE scatter-add spill path.
  * Per 128-token tile, DVE builds notq[tok, j, t] = (wtgt[tok,t] != j) in
    bf16 — j-major layout so all operands are stride-1-packed 2-byte (DVE 2x
    mode); iota is materialized as an uploaded iotarep const.  PE computes
        psw[:, 0:64] += notq_t^T @ feat_t     (= sum(feat) - agg  per target)
        psw[:, 64]   += notq_t^T @ ones       (= 128*T - deg      per target)
        psc          += (-ones)^T @ feat_t    (= -sum(feat), a [1,64] row)
    and a per-window broadcast matmul psw[:,0:64] += onesrow^T @ psc makes
    psw = -agg.  Spill partials (negated on device) merge in with a DVE add.
    mean = psw / min(psd - 2048 - sdeg, -1)  (negative / negative).
  * Window close: PE transpose of mean, then two bf16 GEMMs
    out^T = W_neigh@mean^T + W_self@x^T, ACT adds bias.
  * Software pipelining: window-processing of group g-1 is emitted after the
    gathers+notq of group g, so per-engine streams never head-of-line block
    on cross-engine dependencies.

All heavy DMA is the gather stream: ~200k 256B descriptors per core.
Host-side prep is pure index bookkeeping (bucketing / padding / int16
encoding); all feature math happens on device.
"""
import sys

sys.path.insert(0, "/opt/trn_rl_repo")

import numpy as np


# ---------------------------------------------------------------- config
class _Cfg:
    N = 100000          # nodes
    D = 64              # feature dim
    NCORE = 8
    W = 12500           # targets per core
    NWIN = 98           # 128-target windows per core (98*128 = 12544)
    WPAD = 12544
    NBANK = 4           # source banks of 25000 rows (int16 gather idx limit)
    BANK = 25000
    NCLS = 8            # gather classes: bank * 2 + row parity
    CAP = 256           # bucket capacity (slots per (window, class))
    GW = 7              # windows per gather group
    NG = 14             # groups (NG * GW == NWIN)
    DEAD = 12543        # dead spill cell (padded target, discarded)
    PADTGT = 1000       # wtgt value for pad slots (!= any window-rel target)
    # spill (CCE scatter-add) path
    NSUB = 4            # spill chunks per bank (dup-free round robin)
    SM = 640            # spill chunk capacity (5*128)
    SEW = 68            # spill payload elems: 64 feat + 1 deg + 3 pad

    @property
    def SC(self):
        return self.NWIN * self.CAP          # slots per class (25088)

    @property
    def TOK(self):
        return self.NCLS * self.SC           # total main slots (200704)

    @property
    def CHUNK(self):
        return self.GW * self.CAP            # tokens per (class, group) gather

    @property
    def TPG(self):
        return self.GW * 2                   # tiles per (class, group)

    @property
    def NSCH(self):
        return self.NBANK * self.NSUB        # spill chunks

    @property
    def STOK(self):
        return self.NSCH * self.SM


CFG = _Cfg()


def _build(cfg):
    """Build + compile the SPMD Bass program. Returns the Bacc instance."""
    import concourse.tile as tile
    from concourse import bacc, mybir

    P = 128
    D = cfg.D
    f32 = mybir.dt.float32
    bf16 = mybir.dt.bfloat16
    i16 = mybir.dt.int16
    NE = mybir.AluOpType.not_equal

    nc = bacc.Bacc(None, target_bir_lowering=False, debug=False)
    xpair_d = nc.dram_tensor("xpair", [cfg.N // 2, 2 * D], bf16,
                             kind="ExternalInput")
    x_d = nc.dram_tensor("x", [cfg.N, D], f32, kind="ExternalInput")
    xT_d = nc.dram_tensor("xT", [D, cfg.WPAD], bf16, kind="ExternalInput")
    gidx_d = nc.dram_tensor("gidx", [128, cfg.TOK // 16], i16,
                            kind="ExternalInput")
    wtgt_d = nc.dram_tensor("wtgt", [128, cfg.NCLS * cfg.NWIN * 2], i16,
                            kind="ExternalInput")
    sgidx_d = nc.dram_tensor("sgidx", [128, cfg.STOK // 16], i16,
                             kind="ExternalInput")
    ssidx_d = nc.dram_tensor("ssidx", [128, cfg.STOK // 16], i16,
                             kind="ExternalInput")
    iotar_d = nc.dram_tensor("iotar", [128, 128 * cfg.TPG], i16,
                             kind="ExternalInput")
    ident_d = nc.dram_tensor("ident", [128, 128], bf16, kind="ExternalInput")
    onesb_d = nc.dram_tensor("onesb", [128, 1], bf16, kind="ExternalInput")
    negones_d = nc.dram_tensor("negones", [128, 1], bf16,
                               kind="ExternalInput")
    onesrow_d = nc.dram_tensor("onesrow", [1, 128], bf16,
                               kind="ExternalInput")
    onesrowf_d = nc.dram_tensor("onesrowf", [1, 128], f32,
                                kind="ExternalInput")
    wnT_d = nc.dram_tensor("wnT", [D, D], bf16, kind="ExternalInput")
    wsT_d = nc.dram_tensor("wsT", [D, D], bf16, kind="ExternalInput")
    bias_d = nc.dram_tensor("bias", [D, 1], f32, kind="ExternalInput")
    outT_d = nc.dram_tensor("outT", [D, cfg.WPAD], f32, kind="ExternalOutput")

    GW, TPG, CAP, CHUNK = cfg.GW, cfg.TPG, cfg.CAP, cfg.CHUNK
    NT = cfg.NWIN * 2                        # tiles per class

    with tile.TileContext(nc) as tc:
        with (
            tc.tile_pool(name="const", bufs=1) as cst,
            tc.tile_pool(name="spacc", bufs=1) as spa,
            tc.tile_pool(name="spio", bufs=2) as spio,
            tc.tile_pool(name="gath", bufs=16) as gp,
            tc.tile_pool(name="notq", bufs=16) as qp,
            tc.tile_pool(name="wio", bufs=3) as wio,
            tc.tile_pool(name="wrk", bufs=8) as wrk,
            tc.tile_pool(name="psw", bufs=2, space="PSUM") as pswp,
            tc.tile_pool(name="psd", bufs=2, space="PSUM") as psdp,
            tc.tile_pool(name="psc", bufs=2, space="PSUM") as pscp,
            tc.tile_pool(name="pst", bufs=1, space="PSUM") as pstp,
            tc.tile_pool(name="pso", bufs=1, space="PSUM") as psop,
        ):
            # ---- resident inputs
            gidx_t = cst.tile([128, cfg.TOK // 16], i16)
            wtgt_t = cst.tile([128, cfg.NCLS * NT], i16)
            sgidx_t = cst.tile([128, cfg.STOK // 16], i16)
            ssidx_t = cst.tile([128, cfg.STOK // 16], i16)
            iotar_t = cst.tile([128, 128 * TPG], i16)
            ident_t = cst.tile([128, 128], bf16)
            onesb_t = cst.tile([128, 1], bf16)
            negones_t = cst.tile([128, 1], bf16)
            onesrow_t = cst.tile([1, 128], bf16)
            onesrowf_t = cst.tile([1, 128], f32)
            wnT_t = cst.tile([D, D], bf16)
            wsT_t = cst.tile([D, D], bf16)
            bias_t = cst.tile([D, 1], f32)
            for t, d in ((gidx_t, gidx_d), (wtgt_t, wtgt_d),
                         (sgidx_t, sgidx_d), (ssidx_t, ssidx_d),
                         (iotar_t, iotar_d), (ident_t, ident_d),
                         (onesb_t, onesb_d), (negones_t, negones_d),
                         (onesrow_t, onesrow_d), (onesrowf_t, onesrowf_d),
                         (wnT_t, wnT_d),
                         (wsT_t, wsT_d), (bias_t, bias_d)):
                nc.sync.dma_start(t[:], d[:])

            # ---- spill: CCE scatter-add of negated features + degree
            own_t = spa.tile([P, (cfg.NWIN // 2) * cfg.SEW], f32)
            peer_t = spa.tile([P, (cfg.NWIN // 2) * cfg.SEW], f32)
            nc.vector.memset(own_t[:], 0.0)
            nc.vector.memset(peer_t[:], 0.0)
            SMS = cfg.SM // P
            for ch in range(cfg.NSCH):
                bank = ch // cfg.NSUB
                sl = slice(ch * cfg.SM // 16, (ch + 1) * cfg.SM // 16)
                sp_t = spio.tile([P, SMS * D], f32, name="sp")
                nc.gpsimd.dma_gather(
                    out_ap=sp_t[:].rearrange("p (m e) -> p m e", e=D),
                    in_ap=x_d[bank * cfg.BANK:(bank + 1) * cfg.BANK, :],
                    idxs_ap=sgidx_t[:, sl],
                    num_idxs=cfg.SM,
                    num_idxs_reg=cfg.SM,
                    elem_size=D,
                    single_packet=False,
                )
                aug_t = spio.tile([P, SMS * cfg.SEW], f32, name="aug")
                aug3 = aug_t[:].rearrange("p (m e) -> p m e", e=cfg.SEW)
                nc.vector.memset(aug3[:, :, D:cfg.SEW], -1.0)
                nc.vector.tensor_scalar_mul(
                    aug3[:, :, 0:D],
                    sp_t[:].rearrange("p (m e) -> p m e", e=D), -1.0)
                nc.gpsimd.dma_scatter_add(
                    out_ap=own_t[:],
                    in_ap=aug3,
                    idxs_ap=ssidx_t[:, sl],
                    num_idxs=cfg.SM,
                    num_idxs_reg=cfg.SM,
                    elem_size=cfg.SEW,
                    out_ap_other=peer_t[:],
                    parity_reg=0,
                    sbuf_tokens_per_rank=128,
                    single_packet=False,
                )

            # ---- main: software-pipelined gather/notq (g) + windows (g-1)
            def emit_group_io(g):
                gts = []
                for k in range(cfg.NCLS):
                    bank = k // 2
                    base = k * cfg.SC + g * CHUNK
                    gt = gp.tile([P, TPG * 2 * D], bf16, name="gt")
                    nc.gpsimd.dma_gather(
                        out_ap=gt[:].rearrange("p (m e) -> p m e", e=2 * D),
                        in_ap=xpair_d[bank * (cfg.BANK // 2):
                                      (bank + 1) * (cfg.BANK // 2), :],
                        idxs_ap=gidx_t[:, base // 16:(base + CHUNK) // 16],
                        num_idxs=CHUNK,
                        num_idxs_reg=CHUNK,
                        elem_size=2 * D,
                        single_packet=False,
                    )
                    gts.append(gt)
                qts = []
                for k in range(cfg.NCLS):
                    qt = qp.tile([P, 128 * TPG], bf16, name="qt")
                    wsl = wtgt_t[:, k * NT + g * TPG:k * NT + (g + 1) * TPG]
                    nc.vector.tensor_tensor(
                        qt[:].rearrange("p (j t) -> p j t", t=TPG),
                        wsl.unsqueeze(1).broadcast_to([128, 128, TPG]),
                        iotar_t[:].rearrange("p (j t) -> p j t", t=TPG),
                        NE)
                    qts.append(qt)
                xTg_t = wio.tile([D, GW * P], bf16, name="xTg")
                nc.sync.dma_start(xTg_t[:],
                                  xT_d[:, g * GW * P:(g + 1) * GW * P])
                return gts, qts, xTg_t

            def emit_group_windows(g, gts, qts, xTg_t):
                outg_t = wio.tile([D, GW * P], f32, name="outg")
                for wl in range(GW):
                    w = g * GW + wl
                    psw = pswp.tile([P, D], f32, name="psw")
                    psd = psdp.tile([P, 1], f32, name="psd")
                    psc = pscp.tile([1, D], f32, name="psc")
                    nmm = cfg.NCLS * 2
                    i = 0
                    for k in range(cfg.NCLS):
                        side = k & 1
                        g3 = gts[k][:].rearrange("p (m e) -> p m e", e=2 * D)
                        q3 = qts[k][:].rearrange("p (j t) -> p j t", t=TPG)
                        for h in range(2):
                            c = wl * 2 + h
                            lhsT = q3[:, :, c]
                            rhs = g3[:, c, side * D:(side + 1) * D]
                            st = i == 0
                            sp = i == nmm - 1
                            nc.tensor.matmul(psw[:], lhsT=lhsT, rhs=rhs,
                                             start=st, stop=False)
                            nc.tensor.matmul(psd[:], lhsT=lhsT,
                                             rhs=onesb_t[:],
                                             start=st, stop=sp)
                            nc.tensor.matmul(psc[:], lhsT=negones_t[:],
                                             rhs=rhs, start=st, stop=sp)
                            i += 1
                    # close psw: += onesrow^T @ psc  (broadcast -sum(feat))
                    corr_t = wrk.tile([1, D], f32, name="corr")
                    nc.scalar.copy(corr_t[:], psc[:])
                    nc.tensor.matmul(psw[:], lhsT=onesrowf_t[:],
                                     rhs=corr_t[:], start=False, stop=True)
                    # merge spill (negated partials): psw += spill_feat
                    sacc = own_t if (w & 1) == 0 else peer_t
                    so = (w >> 1) * cfg.SEW
                    nc.vector.tensor_tensor(psw[:], psw[:],
                                            sacc[:, so:so + D],
                                            mybir.AluOpType.add)
                    # -deg_total = (psd - 2048) + (-sdeg); clamp to <= -1
                    ndeg_t = wrk.tile([P, 1], f32, name="nd")
                    nc.vector.scalar_tensor_tensor(
                        ndeg_t[:], psd[:], -2048.0,
                        sacc[:, so + D:so + D + 1],
                        mybir.AluOpType.add, mybir.AluOpType.add)
                    ndegc_t = wrk.tile([P, 1], f32, name="ndc")
                    nc.vector.tensor_scalar_min(ndegc_t[:], ndeg_t[:], -1.0)
                    nrecip_t = wrk.tile([P, 1], f32, name="nrecip")
                    nc.vector.reciprocal(nrecip_t[:], ndegc_t[:])
                    mean_t = wrk.tile([P, D], bf16, name="mean")
                    nc.vector.tensor_scalar_mul(mean_t[:], psw[:],
                                                nrecip_t[:, 0:1])
                    psT = pstp.tile([D, P], bf16, name="psT")
                    nc.tensor.transpose(psT[:], mean_t[:], ident_t[:])
                    meanT_t = wrk.tile([D, P], bf16, name="meanT")
                    nc.scalar.copy(meanT_t[:], psT[:])
                    po = psop.tile([D, P], f32, name="po")
                    nc.tensor.matmul(po[:], lhsT=wnT_t[:], rhs=meanT_t[:],
                                     start=True, stop=False)
                    nc.tensor.matmul(po[:], lhsT=wsT_t[:],
                                     rhs=xTg_t[:, wl * P:(wl + 1) * P],
                                     start=False, stop=True)
                    nc.scalar.activation(outg_t[:, wl * P:(wl + 1) * P],
                                         po[:],
                                         mybir.ActivationFunctionType.Identity,
                                         bias=bias_t[:, 0:1])
                nc.sync.dma_start(outT_d[:, g * GW * P:(g + 1) * GW * P],
                                  outg_t[:])

            pend = None
            for g in range(cfg.NG + 1):
                if g < cfg.NG:
                    gts, qts, xTg_t = emit_group_io(g)
                if pend is not None:
                    emit_group_windows(*pend)
                if g < cfg.NG:
                    pend = (g, gts, qts, xTg_t)

    nc.compile()
    return nc


def _prep_inputs(cfg, x, edge_index, W_self, W_neigh, bias):
    """Host-side index bookkeeping: bucket edges by (window, class) with
    fixed 256-slot caps, route overflow to dup-free spill chunks, int16
    encode, 16-wrap gather indices."""
    import ml_dtypes

    bf16 = ml_dtypes.bfloat16
    x = np.ascontiguousarray(np.asarray(x, dtype=np.float32))
    ei = np.asarray(edge_index)
    row = ei[0].astype(np.int64)
    col = ei[1].astype(np.int64)

    xpair = np.ascontiguousarray(
        x.astype(bf16).reshape(cfg.N // 2, 2 * cfg.D))
    wnT = np.ascontiguousarray(np.asarray(W_neigh, np.float32).T.astype(bf16))
    wsT = np.ascontiguousarray(np.asarray(W_self, np.float32).T.astype(bf16))
    bias_c = np.ascontiguousarray(
        np.asarray(bias, np.float32).reshape(cfg.D, 1))
    iotar = np.ascontiguousarray(np.tile(
        np.repeat(np.arange(128, dtype=np.int16), cfg.TPG), (128, 1)))
    ident = np.ascontiguousarray(np.eye(128, dtype=np.float32).astype(bf16))
    onesb = np.ones((128, 1), dtype=bf16)
    negones = np.full((128, 1), -1.0, dtype=bf16)
    onesrow = np.ones((1, 128), dtype=bf16)

    def wrap16(a):
        return np.tile(np.ascontiguousarray(a.reshape(-1, 16).T), (8, 1))

    owner = col // cfg.W
    in_maps = []
    for c in range(cfg.NCORE):
        msk = owner == c
        r = row[msk]
        tl = (col[msk] - c * cfg.W).astype(np.int64)
        w = tl >> 7
        j = tl & 127
        bank = r // cfg.BANK
        L = r % cfg.BANK
        side = L & 1
        g16 = L >> 1
        cls = bank * 2 + side
        bucket = cls * cfg.NWIN + w

        # occurrence index within (bucket, target)
        n = len(tl)
        o1 = np.lexsort((tl, bucket))
        bt = bucket[o1] * 16384 + tl[o1]
        first = np.r_[True, bt[1:] != bt[:-1]]
        runstart = np.maximum.accumulate(np.where(first, np.arange(n), 0))
        occ_s = np.arange(n) - runstart
        occ = np.empty(n, np.int64)
        occ[o1] = occ_s

        # rank within bucket by occ DESCENDING -> first CAP stay, rest spill.
        # Main buckets tolerate any per-target multiplicity (matmul segsum),
        # spill chunks do not (CCE RMW races) -- so spill the occ-0 tokens,
        # which are distinct targets by construction.
        o2 = np.lexsort((-occ, bucket))
        b_s = bucket[o2]
        first_b = np.r_[True, b_s[1:] != b_s[:-1]]
        runstart_b = np.maximum.accumulate(np.where(first_b, np.arange(n), 0))
        rank_s = np.arange(n) - runstart_b
        rank = np.empty(n, np.int64)
        rank[o2] = rank_s

        main = rank < cfg.CAP
        slot = cls[main] * cfg.SC + w[main] * cfg.CAP + rank[main]
        gbuf = np.zeros(cfg.TOK, np.int16)
        tbuf = np.full(cfg.TOK, cfg.PADTGT, np.int16)
        gbuf[slot] = g16[main].astype(np.int16)
        tbuf[slot] = j[main].astype(np.int16)

        # spill: per (bank, target) round-robin over NSUB chunks
        sm = ~main
        sb = bank[sm]
        sL = L[sm]
        stl = tl[sm]
        ns = len(stl)
        sgbuf = np.zeros(cfg.STOK, np.int16)
        ssbuf = np.full(cfg.STOK, cfg.DEAD, np.int16)
        if ns:
            o3 = np.lexsort((stl, sb))
            key = sb[o3] * 16384 + stl[o3]
            f3 = np.r_[True, key[1:] != key[:-1]]
            rs3 = np.maximum.accumulate(np.where(f3, np.arange(ns), 0))
            occ3_s = np.arange(ns) - rs3
            occ3 = np.empty(ns, np.int64)
            occ3[o3] = occ3_s
            if occ3.max() >= cfg.NSUB:
                raise RuntimeError(
                    f"spill multiplicity {occ3.max() + 1} > NSUB={cfg.NSUB}")
            sub = (occ3 + stl * 7) % cfg.NSUB
            chunk = sb * cfg.NSUB + sub
            o4 = np.lexsort((np.arange(ns), chunk))
            ch_s = chunk[o4]
            f4 = np.r_[True, ch_s[1:] != ch_s[:-1]]
            rs4 = np.maximum.accumulate(np.where(f4, np.arange(ns), 0))
            pos_s = np.arange(ns) - rs4
            pos = np.empty(ns, np.int64)
            pos[o4] = pos_s
            if pos.max() >= cfg.SM:
                raise RuntimeError(
                    f"spill chunk overflow: {pos.max() + 1} > SM={cfg.SM}")
            sslot = chunk * cfg.SM + pos
            sgbuf[sslot] = sL.astype(np.int16)
            ssbuf[sslot] = stl.astype(np.int16)

        # wtgt: [128, NCLS * NWIN * 2], column = class-major tile index,
        # wtgt[p, t] = window-relative target of slot t*128 + p
        wtgt = np.ascontiguousarray(
            tbuf.reshape(cfg.NCLS * cfg.NWIN * 2, 128).T)

        xw = np.zeros((cfg.D, cfg.WPAD), np.float32)
        xw[:, :cfg.W] = x[c * cfg.W:(c + 1) * cfg.W].T
        in_maps.append({
            "xpair": xpair,
            "x": x,
            "xT": np.ascontiguousarray(xw.astype(bf16)),
            "gidx": wrap16(gbuf),
            "wtgt": wtgt,
            "sgidx": wrap16(sgbuf),
            "ssidx": wrap16(ssbuf),
            "iotar": iotar,
            "ident": ident,
            "onesb": onesb,
            "negones": negones,
            "onesrow": onesrow,
            "onesrowf": np.ones((1, 128), np.float32),
            "wnT": wnT,
            "wsT": wsT,
            "bias": bias_c,
        })
    return in_maps


_CACHED_NC = None


def _get_nc():
    global _CACHED_NC
    if _CACHED_NC is None:
        _CACHED_NC = _build(CFG)
    return _CACHED_NC


def kernel(x, edge_index, W_self, W_neigh, bias, _trace=False,
           _trace_kwargs=None):
    from concourse.bass_utils import run_bass_kernel_spmd

    cfg = CFG
    nc = _get_nc()
    in_maps = _prep_inputs(cfg, x, edge_index, W_self, W_neigh, bias)
    kw = {}
    if _trace:
        kw["trace"] = True
        if _trace_kwargs:
            kw.update(_trace_kwargs)
    res = run_bass_kernel_spmd(nc, in_maps, list(range(cfg.NCORE)), **kw)
    out = np.concatenate(
        [res.results[c]["outT"][:, :cfg.W].T for c in range(cfg.NCORE)],
        axis=0)
    out = np.ascontiguousarray(out, dtype=np.float32)
    if _trace:
        return out, res
    return out


# revision 4
# speedup vs baseline: 1.0381x; 1.0381x over previous
"""DimGraphConv (GNN mean-aggregation message passing) on 8 Trainium2 cores.

Math (per reference):
    out = x @ W_self.T + segment_mean(x[row], col) @ W_neigh.T + bias

v3 design: target-sharded (core c owns targets [c*12500, (c+1)*12500)); the
per-edge aggregation runs on the TENSOR engine instead of the CCE scatter-add:

  * Edges are bucketed by (128-target window w, gather class k).  A class is
    (source bank, row parity): x is stored as a bf16 row-pair array
    xpair[50000, 128] so one 256B gather descriptor fetches the row pair that
    contains the wanted source row; parity picks the 64-element half.
  * Each bucket has a fixed 256-slot capacity (2 matmul tiles).  Overflow
    (~2.5% of edges) is routed to a small CCE scatter-add spill path.
  * Per 128-token tile, DVE builds notq[tok, j, t] = (wtgt[tok,t] != j) in
    bf16 — j-major layout so all operands are stride-1-packed 2-byte (DVE 2x
    mode); iota is materialized as an uploaded iotarep const.  PE computes
        psw[:, 0:64] += notq_t^T @ feat_t     (= sum(feat) - agg  per target)
        psw[:, 64]   += notq_t^T @ ones       (= 128*T - deg      per target)
        psc          += (-ones)^T @ feat_t    (= -sum(feat), a [1,64] row)
    and a per-window broadcast matmul psw[:,0:64] += onesrow^T @ psc makes
    psw = -agg.  Spill partials (negated on device) merge in with a DVE add.
    mean = psw / min(psd - 2048 - sdeg, -1)  (negative / negative).
  * Window close: PE transpose of mean, then two bf16 GEMMs
    out^T = W_neigh@mean^T + W_self@x^T, ACT adds bias.
  * Software pipelining: window-processing of group g-1 is emitted after the
    gathers+notq of group g, so per-engine streams never head-of-line block
    on cross-engine dependencies.

All heavy DMA is the gather stream: ~200k 256B descriptors per core.
Host-side prep is pure index bookkeeping (bucketing / padding / int16
encoding); all feature math happens on device.
"""
import sys

sys.path.insert(0, "/opt/trn_rl_repo")

import numpy as np


# ---------------------------------------------------------------- config
class _Cfg:
    N = 100000          # nodes
    D = 64              # feature dim
    NCORE = 8
    W = 12500           # targets per core
    NWIN = 98           # 128-target windows per core (98*128 = 12544)
    WPAD = 12544
    NBANK = 4           # source banks of 25000 rows (int16 gather idx limit)
    BANK = 25000
    NCLS = 8            # gather classes: bank * 2 + row parity
    CAP = 256           # bucket capacity (slots per (window, class))
    GW = 7              # windows per gather group
    NG = 14             # groups (NG * GW == NWIN)
    DEAD = 12543        # dead spill cell (padded target, discarded)
    PADTGT = 1000       # wtgt value for pad slots (!= any window-rel target)
    # spill (CCE scatter-add) path
    NSUB = 2            # spill chunks per bank (dup-free round robin)
    SM = 896            # spill chunk capacity (7*128)
    SEW = 68            # spill payload elems: 64 feat + 1 deg + 3 pad

    @property
    def SC(self):
        return self.NWIN * self.CAP          # slots per class (25088)

    @property
    def TOK(self):
        return self.NCLS * self.SC           # total main slots (200704)

    @property
    def CHUNK(self):
        return self.GW * self.CAP            # tokens per (class, group) gather

    @property
    def TPG(self):
        return self.GW * 2                   # tiles per (class, group)

    @property
    def NSCH(self):
        return self.NBANK * self.NSUB        # spill chunks

    @property
    def STOK(self):
        return self.NSCH * self.SM


CFG = _Cfg()


def _build(cfg):
    """Build + compile the SPMD Bass program. Returns the Bacc instance."""
    import concourse.tile as tile
    from concourse import bacc, mybir

    P = 128
    D = cfg.D
    f32 = mybir.dt.float32
    bf16 = mybir.dt.bfloat16
    i16 = mybir.dt.int16
    NE = mybir.AluOpType.not_equal

    nc = bacc.Bacc(None, target_bir_lowering=False, debug=False)
    xpair_d = nc.dram_tensor("xpair", [cfg.N // 2, 2 * D], bf16,
                             kind="ExternalInput")
    x_d = nc.dram_tensor("x", [cfg.N, D], f32, kind="ExternalInput")
    xT_d = nc.dram_tensor("xT", [D, cfg.WPAD], bf16, kind="ExternalInput")
    gidx_d = nc.dram_tensor("gidx", [128, cfg.TOK // 16], i16,
                            kind="ExternalInput")
    wtgt_d = nc.dram_tensor("wtgt", [128, cfg.NCLS * cfg.NWIN * 2], i16,
                            kind="ExternalInput")
    sgidx_d = nc.dram_tensor("sgidx", [128, cfg.STOK // 16], i16,
                             kind="ExternalInput")
    ssidx_d = nc.dram_tensor("ssidx", [128, cfg.STOK // 16], i16,
                             kind="ExternalInput")
    iotar_d = nc.dram_tensor("iotar", [128, 128 * cfg.TPG], i16,
                             kind="ExternalInput")
    ident_d = nc.dram_tensor("ident", [128, 128], bf16, kind="ExternalInput")
    onesb_d = nc.dram_tensor("onesb", [128, 1], bf16, kind="ExternalInput")
    negones_d = nc.dram_tensor("negones", [128, 1], bf16,
                               kind="ExternalInput")
    onesrow_d = nc.dram_tensor("onesrow", [1, 128], bf16,
                               kind="ExternalInput")
    onesrowf_d = nc.dram_tensor("onesrowf", [1, 128], f32,
                                kind="ExternalInput")
    wnT_d = nc.dram_tensor("wnT", [D, D], bf16, kind="ExternalInput")
    wsT_d = nc.dram_tensor("wsT", [D, D], bf16, kind="ExternalInput")
    bias_d = nc.dram_tensor("bias", [D, 1], f32, kind="ExternalInput")
    outT_d = nc.dram_tensor("outT", [D, cfg.WPAD], f32, kind="ExternalOutput")

    GW, TPG, CAP, CHUNK = cfg.GW, cfg.TPG, cfg.CAP, cfg.CHUNK
    NT = cfg.NWIN * 2                        # tiles per class

    with tile.TileContext(nc) as tc:
        with (
            tc.tile_pool(name="const", bufs=1) as cst,
            tc.tile_pool(name="spacc", bufs=1) as spa,
            tc.tile_pool(name="gath", bufs=16) as gp,
            tc.tile_pool(name="notq", bufs=16) as qp,
            tc.tile_pool(name="wio", bufs=3) as wio,
            tc.tile_pool(name="wrk", bufs=8) as wrk,
            tc.tile_pool(name="psw", bufs=3, space="PSUM") as pswp,
            tc.tile_pool(name="psd", bufs=1, space="PSUM") as psdp,
            tc.tile_pool(name="psc", bufs=2, space="PSUM") as pscp,
            tc.tile_pool(name="pst", bufs=1, space="PSUM") as pstp,
            tc.tile_pool(name="pso", bufs=1, space="PSUM") as psop,
        ):
            # ---- resident inputs
            gidx_t = cst.tile([128, cfg.TOK // 16], i16)
            wtgt_t = cst.tile([128, cfg.NCLS * NT], i16)
            sgidx_t = cst.tile([128, cfg.STOK // 16], i16)
            ssidx_t = cst.tile([128, cfg.STOK // 16], i16)
            iotar_t = cst.tile([128, 128 * TPG], i16)
            ident_t = cst.tile([128, 128], bf16)
            onesb_t = cst.tile([128, 1], bf16)
            negones_t = cst.tile([128, 1], bf16)
            onesrow_t = cst.tile([1, 128], bf16)
            onesrowf_t = cst.tile([1, 128], f32)
            wnT_t = cst.tile([D, D], bf16)
            wsT_t = cst.tile([D, D], bf16)
            bias_t = cst.tile([D, 1], f32)
            for t, d in ((gidx_t, gidx_d), (wtgt_t, wtgt_d),
                         (sgidx_t, sgidx_d), (ssidx_t, ssidx_d),
                         (iotar_t, iotar_d), (ident_t, ident_d),
                         (onesb_t, onesb_d), (negones_t, negones_d),
                         (onesrow_t, onesrow_d), (onesrowf_t, onesrowf_d),
                         (wnT_t, wnT_d),
                         (wsT_t, wsT_d), (bias_t, bias_d)):
                nc.sync.dma_start(t[:], d[:])

            # ---- spill: CCE scatter-add of negated features + degree
            own_t = spa.tile([P, (cfg.NWIN // 2) * cfg.SEW], f32)
            peer_t = spa.tile([P, (cfg.NWIN // 2) * cfg.SEW], f32)
            nc.vector.memset(own_t[:], 0.0)
            nc.vector.memset(peer_t[:], 0.0)
            SMS = cfg.SM // P
            spio_cm = tc.tile_pool(name="spio", bufs=3)
            spio = spio_cm.__enter__()
            sp_ts = []
            for ch in range(cfg.NSCH):
                bank = ch // cfg.NSUB
                sl = slice(ch * cfg.SM // 16, (ch + 1) * cfg.SM // 16)
                sp_t = spio.tile([P, SMS * D], f32, name="sp")
                nc.gpsimd.dma_gather(
                    out_ap=sp_t[:].rearrange("p (m e) -> p m e", e=D),
                    in_ap=x_d[bank * cfg.BANK:(bank + 1) * cfg.BANK, :],
                    idxs_ap=sgidx_t[:, sl],
                    num_idxs=cfg.SM,
                    num_idxs_reg=cfg.SM,
                    elem_size=D,
                    single_packet=False,
                )
                sp_ts.append(sp_t)
            aug_ts = []
            for ch in range(cfg.NSCH):
                aug_t = spio.tile([P, SMS * cfg.SEW], f32, name="aug")
                aug3 = aug_t[:].rearrange("p (m e) -> p m e", e=cfg.SEW)
                nc.vector.memset(aug3[:, :, D:cfg.SEW], -1.0)
                nc.vector.tensor_scalar_mul(
                    aug3[:, :, 0:D],
                    sp_ts[ch][:].rearrange("p (m e) -> p m e", e=D), -1.0)
                aug_ts.append(aug3)
            for ch in range(cfg.NSCH):
                sl = slice(ch * cfg.SM // 16, (ch + 1) * cfg.SM // 16)
                nc.gpsimd.dma_scatter_add(
                    out_ap=own_t[:],
                    in_ap=aug_ts[ch],
                    idxs_ap=ssidx_t[:, sl],
                    num_idxs=cfg.SM,
                    num_idxs_reg=cfg.SM,
                    elem_size=cfg.SEW,
                    out_ap_other=peer_t[:],
                    parity_reg=0,
                    sbuf_tokens_per_rank=128,
                    single_packet=False,
                )

            spio_cm.__exit__(None, None, None)

            # ---- main: software-pipelined gather/notq (g) + windows (g-1)
            def emit_group_io(wbase, gw):
                tpg = gw * 2
                chunk = gw * CAP
                gts = []
                for k in range(cfg.NCLS):
                    bank = k // 2
                    base = k * cfg.SC + wbase * CAP
                    gt = gp.tile([P, TPG * 2 * D], bf16, name="gt")
                    nc.gpsimd.dma_gather(
                        out_ap=gt[:, :tpg * 2 * D].rearrange(
                            "p (m e) -> p m e", e=2 * D),
                        in_ap=xpair_d[bank * (cfg.BANK // 2):
                                      (bank + 1) * (cfg.BANK // 2), :],
                        idxs_ap=gidx_t[:, base // 16:(base + chunk) // 16],
                        num_idxs=chunk,
                        num_idxs_reg=chunk,
                        elem_size=2 * D,
                        single_packet=False,
                    )
                    gts.append(gt)
                qts = []
                for k in range(cfg.NCLS):
                    qt = qp.tile([P, 128 * TPG], bf16, name="qt")
                    wsl = wtgt_t[:, k * NT + wbase * 2:
                                 k * NT + wbase * 2 + tpg]
                    nc.vector.tensor_tensor(
                        qt[:, :128 * tpg].rearrange("p (j t) -> p j t",
                                                    t=tpg),
                        wsl.unsqueeze(1).broadcast_to([128, 128, tpg]),
                        iotar_t[:].rearrange("p (j t) -> p j t",
                                             t=TPG)[:, :, 0:tpg],
                        NE)
                    qts.append(qt)
                xTg_t = wio.tile([D, GW * P], bf16, name="xTg")
                nc.sync.dma_start(xTg_t[:, :gw * P],
                                  xT_d[:, wbase * P:(wbase + gw) * P])
                return gts, qts, xTg_t

            def emit_group_windows(wbase, gw, gts, qts, xTg_t):
                tpg = gw * 2
                outg_t = wio.tile([D, GW * P], f32, name="outg")
                for wl in range(gw):
                    w = wbase + wl
                    psw = pswp.tile([P, D], f32, name="psw")
                    psd = psdp.tile([P, 1], f32, name="psd")
                    psc = pscp.tile([1, D], f32, name="psc")
                    nmm = cfg.NCLS * 2
                    i = 0
                    for k in range(cfg.NCLS):
                        side = k & 1
                        g3 = gts[k][:, :tpg * 2 * D].rearrange(
                            "p (m e) -> p m e", e=2 * D)
                        q3 = qts[k][:, :128 * tpg].rearrange(
                            "p (j t) -> p j t", t=tpg)
                        for h in range(2):
                            c = wl * 2 + h
                            lhsT = q3[:, :, c]
                            rhs = g3[:, c, side * D:(side + 1) * D]
                            st = i == 0
                            sp = i == nmm - 1
                            nc.tensor.matmul(psw[:], lhsT=lhsT, rhs=rhs,
                                             start=st, stop=False)
                            nc.tensor.matmul(psd[:], lhsT=lhsT,
                                             rhs=onesb_t[:],
                                             start=st, stop=sp)
                            nc.tensor.matmul(psc[:], lhsT=negones_t[:],
                                             rhs=rhs, start=st, stop=sp)
                            i += 1
                    # close psw: += onesrow^T @ psc  (broadcast -sum(feat))
                    corr_t = wrk.tile([1, D], f32, name="corr")
                    nc.scalar.copy(corr_t[:], psc[:])
                    nc.tensor.matmul(psw[:], lhsT=onesrowf_t[:],
                                     rhs=corr_t[:], start=False, stop=True)
                    # merge spill (negated partials): psw += spill_feat
                    sacc = own_t if (w & 1) == 0 else peer_t
                    so = (w >> 1) * cfg.SEW
                    nc.vector.tensor_tensor(psw[:], psw[:],
                                            sacc[:, so:so + D],
                                            mybir.AluOpType.add)
                    # -deg_total = (psd - 2048) + (-sdeg); clamp to <= -1
                    ndeg_t = wrk.tile([P, 1], f32, name="nd")
                    nc.vector.scalar_tensor_tensor(
                        ndeg_t[:], psd[:], -2048.0,
                        sacc[:, so + D:so + D + 1],
                        mybir.AluOpType.add, mybir.AluOpType.add)
                    ndegc_t = wrk.tile([P, 1], f32, name="ndc")
                    nc.vector.tensor_scalar_min(ndegc_t[:], ndeg_t[:], -1.0)
                    nrecip_t = wrk.tile([P, 1], f32, name="nrecip")
                    nc.vector.reciprocal(nrecip_t[:], ndegc_t[:])
                    mean_t = wrk.tile([P, D], bf16, name="mean")
                    nc.vector.tensor_scalar_mul(mean_t[:], psw[:],
                                                nrecip_t[:, 0:1])
                    psT = pstp.tile([D, P], bf16, name="psT")
                    nc.tensor.transpose(psT[:], mean_t[:], ident_t[:])
                    meanT_t = wrk.tile([D, P], bf16, name="meanT")
                    nc.scalar.copy(meanT_t[:], psT[:])
                    po = psop.tile([D, P], f32, name="po")
                    nc.tensor.matmul(po[:], lhsT=wnT_t[:], rhs=meanT_t[:],
                                     start=True, stop=False)
                    nc.tensor.matmul(po[:], lhsT=wsT_t[:],
                                     rhs=xTg_t[:, wl * P:(wl + 1) * P],
                                     start=False, stop=True)
                    nc.scalar.activation(outg_t[:, wl * P:(wl + 1) * P],
                                         po[:],
                                         mybir.ActivationFunctionType.Identity,
                                         bias=bias_t[:, 0:1])
                nc.sync.dma_start(outT_d[:, wbase * P:(wbase + gw) * P],
                                  outg_t[:, :gw * P])

            groups = [GW] * 12 + [5, 4, 3, 2]
            assert sum(groups) == cfg.NWIN
            pend = None
            wbase = 0
            for gi in range(len(groups) + 1):
                if gi < len(groups):
                    gw = groups[gi]
                    gts, qts, xTg_t = emit_group_io(wbase, gw)
                if pend is not None:
                    emit_group_windows(*pend)
                if gi < len(groups):
                    pend = (wbase, gw, gts, qts, xTg_t)
                    wbase += gw

    nc.compile()
    return nc


def _prep_inputs(cfg, x, edge_index, W_self, W_neigh, bias):
    """Host-side index bookkeeping: bucket edges by (window, class) with
    fixed 256-slot caps, route overflow to dup-free spill chunks, int16
    encode, 16-wrap gather indices."""
    import ml_dtypes

    bf16 = ml_dtypes.bfloat16
    x = np.ascontiguousarray(np.asarray(x, dtype=np.float32))
    ei = np.asarray(edge_index)
    row = ei[0].astype(np.int64)
    col = ei[1].astype(np.int64)

    xpair = np.ascontiguousarray(
        x.astype(bf16).reshape(cfg.N // 2, 2 * cfg.D))
    wnT = np.ascontiguousarray(np.asarray(W_neigh, np.float32).T.astype(bf16))
    wsT = np.ascontiguousarray(np.asarray(W_self, np.float32).T.astype(bf16))
    bias_c = np.ascontiguousarray(
        np.asarray(bias, np.float32).reshape(cfg.D, 1))
    iotar = np.ascontiguousarray(np.tile(
        np.repeat(np.arange(128, dtype=np.int16), cfg.TPG), (128, 1)))
    ident = np.ascontiguousarray(np.eye(128, dtype=np.float32).astype(bf16))
    onesb = np.ones((128, 1), dtype=bf16)
    negones = np.full((128, 1), -1.0, dtype=bf16)
    onesrow = np.ones((1, 128), dtype=bf16)

    def wrap16(a):
        return np.tile(np.ascontiguousarray(a.reshape(-1, 16).T), (8, 1))

    owner = col // cfg.W
    in_maps = []
    for c in range(cfg.NCORE):
        msk = owner == c
        r = row[msk]
        tl = (col[msk] - c * cfg.W).astype(np.int64)
        w = tl >> 7
        j = tl & 127
        bank = r // cfg.BANK
        L = r % cfg.BANK
        side = L & 1
        g16 = L >> 1
        cls = bank * 2 + side
        bucket = cls * cfg.NWIN + w

        # occurrence index within (bucket, target)
        n = len(tl)
        o1 = np.lexsort((tl, bucket))
        bt = bucket[o1] * 16384 + tl[o1]
        first = np.r_[True, bt[1:] != bt[:-1]]
        runstart = np.maximum.accumulate(np.where(first, np.arange(n), 0))
        occ_s = np.arange(n) - runstart
        occ = np.empty(n, np.int64)
        occ[o1] = occ_s

        # rank within bucket by occ DESCENDING -> first CAP stay, rest spill.
        # Main buckets tolerate any per-target multiplicity (matmul segsum),
        # spill chunks do not (CCE RMW races) -- so spill the occ-0 tokens,
        # which are distinct targets by construction.
        o2 = np.lexsort((-occ, bucket))
        b_s = bucket[o2]
        first_b = np.r_[True, b_s[1:] != b_s[:-1]]
        runstart_b = np.maximum.accumulate(np.where(first_b, np.arange(n), 0))
        rank_s = np.arange(n) - runstart_b
        rank = np.empty(n, np.int64)
        rank[o2] = rank_s

        main = rank < cfg.CAP
        slot = cls[main] * cfg.SC + w[main] * cfg.CAP + rank[main]
        gbuf = np.zeros(cfg.TOK, np.int16)
        tbuf = np.full(cfg.TOK, cfg.PADTGT, np.int16)
        gbuf[slot] = g16[main].astype(np.int16)
        tbuf[slot] = j[main].astype(np.int16)

        # spill: per (bank, target) round-robin over NSUB chunks
        sm = ~main
        sb = bank[sm]
        sL = L[sm]
        stl = tl[sm]
        ns = len(stl)
        sgbuf = np.zeros(cfg.STOK, np.int16)
        ssbuf = np.full(cfg.STOK, cfg.DEAD, np.int16)
        if ns:
            o3 = np.lexsort((stl, sb))
            key = sb[o3] * 16384 + stl[o3]
            f3 = np.r_[True, key[1:] != key[:-1]]
            rs3 = np.maximum.accumulate(np.where(f3, np.arange(ns), 0))
            occ3_s = np.arange(ns) - rs3
            occ3 = np.empty(ns, np.int64)
            occ3[o3] = occ3_s
            if occ3.max() >= cfg.NSUB:
                raise RuntimeError(
                    f"spill multiplicity {occ3.max() + 1} > NSUB={cfg.NSUB}")
            sub = (occ3 + ((stl * 2654435761) >> 8)) % cfg.NSUB
            chunk = sb * cfg.NSUB + sub
            o4 = np.lexsort((np.arange(ns), chunk))
            ch_s = chunk[o4]
            f4 = np.r_[True, ch_s[1:] != ch_s[:-1]]
            rs4 = np.maximum.accumulate(np.where(f4, np.arange(ns), 0))
            pos_s = np.arange(ns) - rs4
            pos = np.empty(ns, np.int64)
            pos[o4] = pos_s
            if pos.max() >= cfg.SM:
                raise RuntimeError(
                    f"spill chunk overflow: {pos.max() + 1} > SM={cfg.SM}")
            sslot = chunk * cfg.SM + pos
            sgbuf[sslot] = sL.astype(np.int16)
            ssbuf[sslot] = stl.astype(np.int16)

        # wtgt: [128, NCLS * NWIN * 2], column = class-major tile index,
        # wtgt[p, t] = window-relative target of slot t*128 + p
        wtgt = np.ascontiguousarray(
            tbuf.reshape(cfg.NCLS * cfg.NWIN * 2, 128).T)

        xw = np.zeros((cfg.D, cfg.WPAD), np.float32)
        xw[:, :cfg.W] = x[c * cfg.W:(c + 1) * cfg.W].T
        in_maps.append({
            "xpair": xpair,
            "x": x,
            "xT": np.ascontiguousarray(xw.astype(bf16)),
            "gidx": wrap16(gbuf),
            "wtgt": wtgt,
            "sgidx": wrap16(sgbuf),
            "ssidx": wrap16(ssbuf),
            "iotar": iotar,
            "ident": ident,
            "onesb": onesb,
            "negones": negones,
            "onesrow": onesrow,
            "onesrowf": np.ones((1, 128), np.float32),
            "wnT": wnT,
            "wsT": wsT,
            "bias": bias_c,
        })
    return in_maps


_CACHED_NC = None


def _get_nc():
    global _CACHED_NC
    if _CACHED_NC is None:
        _CACHED_NC = _build(CFG)
    return _CACHED_NC


def kernel(x, edge_index, W_self, W_neigh, bias, _trace=False,
           _trace_kwargs=None):
    from concourse.bass_utils import run_bass_kernel_spmd

    cfg = CFG
    nc = _get_nc()
    in_maps = _prep_inputs(cfg, x, edge_index, W_self, W_neigh, bias)
    kw = {}
    if _trace:
        kw["trace"] = True
        if _trace_kwargs:
            kw.update(_trace_kwargs)
    res = run_bass_kernel_spmd(nc, in_maps, list(range(cfg.NCORE)), **kw)
    out = np.concatenate(
        [res.results[c]["outT"][:, :cfg.W].T for c in range(cfg.NCORE)],
        axis=0)
    out = np.ascontiguousarray(out, dtype=np.float32)
    if _trace:
        return out, res
    return out


# revision 7
# speedup vs baseline: 1.0695x; 1.0302x over previous
"""DimGraphConv (GNN mean-aggregation message passing) on 8 Trainium2 cores.

Math (per reference):
    out = x @ W_self.T + segment_mean(x[row], col) @ W_neigh.T + bias

v3 design: target-sharded (core c owns targets [c*12500, (c+1)*12500)); the
per-edge aggregation runs on the TENSOR engine instead of the CCE scatter-add:

  * Edges are bucketed by (128-target window w, gather class k).  A class is
    (source bank, row parity): x is stored as a bf16 row-pair array
    xpair[50000, 128] so one 256B gather descriptor fetches the row pair that
    contains the wanted source row; parity picks the 64-element half.
  * Each bucket has a fixed 256-slot capacity (2 matmul tiles).  Overflow
    (~2.5% of edges) is routed to a small CCE scatter-add spill path.
  * Per 128-token tile, DVE builds notq[tok, j, t] = (wtgt[tok,t] != j) in
    bf16 — j-major layout so all operands are stride-1-packed 2-byte (DVE 2x
    mode); iota is materialized as an uploaded iotarep const.  PE computes
        psw[:, 0:64] += notq_t^T @ feat_t     (= sum(feat) - agg  per target)
        psw[:, 64]   += notq_t^T @ ones       (= 128*T - deg      per target)
        psc          += (-ones)^T @ feat_t    (= -sum(feat), a [1,64] row)
    and a per-window broadcast matmul psw[:,0:64] += onesrow^T @ psc makes
    psw = -agg.  Spill partials (negated on device) merge in with a DVE add.
    mean = psw / min(psd - 2048 - sdeg, -1)  (negative / negative).
  * Window close: PE transpose of mean, then two bf16 GEMMs
    out^T = W_neigh@mean^T + W_self@x^T, ACT adds bias.
  * Software pipelining: window-processing of group g-1 is emitted after the
    gathers+notq of group g, so per-engine streams never head-of-line block
    on cross-engine dependencies.

All heavy DMA is the gather stream: ~200k 256B descriptors per core.
Host-side prep is pure index bookkeeping (bucketing / padding / int16
encoding); all feature math happens on device.
"""
import sys

sys.path.insert(0, "/opt/trn_rl_repo")

import numpy as np


# ---------------------------------------------------------------- config
class _Cfg:
    N = 100000          # nodes
    D = 64              # feature dim
    NCORE = 8
    W = 12500           # targets per core
    NWIN = 98           # 128-target windows per core (98*128 = 12544)
    WPAD = 12544
    NBANK = 4           # source banks of 25000 rows (int16 gather idx limit)
    BANK = 25000
    NCLS = 8            # gather classes: bank * 2 + row parity
    CAP = 256           # bucket capacity (slots per (window, class))
    GW = 7              # windows per gather group
    NG = 14             # groups (NG * GW == NWIN)
    DEAD = 12543        # dead spill cell (padded target, discarded)
    PADTGT = 1000       # wtgt value for pad slots (!= any window-rel target)
    # spill (CCE scatter-add) path
    NSUB = 2            # spill chunks per bank (dup-free round robin)
    SM = 896            # spill chunk capacity (7*128)
    SEW = 68            # spill payload elems: 64 feat + 1 deg + 3 pad

    @property
    def SC(self):
        return self.NWIN * self.CAP          # slots per class (25088)

    @property
    def TOK(self):
        return self.NCLS * self.SC           # total main slots (200704)

    @property
    def CHUNK(self):
        return self.GW * self.CAP            # tokens per (class, group) gather

    @property
    def TPG(self):
        return self.GW * 2                   # tiles per (class, group)

    @property
    def NSCH(self):
        return self.NBANK * self.NSUB        # spill chunks

    @property
    def STOK(self):
        return self.NSCH * self.SM


CFG = _Cfg()


def _build(cfg):
    """Build + compile the SPMD Bass program. Returns the Bacc instance."""
    import concourse.tile as tile
    from concourse import bacc, mybir

    P = 128
    D = cfg.D
    f32 = mybir.dt.float32
    bf16 = mybir.dt.bfloat16
    i16 = mybir.dt.int16
    NE = mybir.AluOpType.not_equal

    nc = bacc.Bacc(None, target_bir_lowering=False, debug=False)
    xpair_d = nc.dram_tensor("xpair", [cfg.N // 2, 2 * D], bf16,
                             kind="ExternalInput")
    x_d = nc.dram_tensor("x", [cfg.N, D], f32, kind="ExternalInput")
    xT_d = nc.dram_tensor("xT", [D, cfg.WPAD], bf16, kind="ExternalInput")
    gidx_d = nc.dram_tensor("gidx", [128, cfg.TOK // 16], i16,
                            kind="ExternalInput")
    wtgt_d = nc.dram_tensor("wtgt", [128, cfg.NCLS * cfg.NWIN * 2], i16,
                            kind="ExternalInput")
    sgidx_d = nc.dram_tensor("sgidx", [128, cfg.STOK // 16], i16,
                             kind="ExternalInput")
    ssidx_d = nc.dram_tensor("ssidx", [128, cfg.STOK // 16], i16,
                             kind="ExternalInput")
    iotar_d = nc.dram_tensor("iotar", [128, 128 * cfg.TPG], i16,
                             kind="ExternalInput")
    ident_d = nc.dram_tensor("ident", [128, 128], bf16, kind="ExternalInput")
    onesb_d = nc.dram_tensor("onesb", [128, 1], bf16, kind="ExternalInput")
    negones_d = nc.dram_tensor("negones", [128, 1], bf16,
                               kind="ExternalInput")
    onesrow_d = nc.dram_tensor("onesrow", [1, 128], bf16,
                               kind="ExternalInput")
    onesrowf_d = nc.dram_tensor("onesrowf", [1, 128], f32,
                                kind="ExternalInput")
    wnT_d = nc.dram_tensor("wnT", [D, D], bf16, kind="ExternalInput")
    wsT_d = nc.dram_tensor("wsT", [D, D], bf16, kind="ExternalInput")
    bias_d = nc.dram_tensor("bias", [D, 1], f32, kind="ExternalInput")
    outT_d = nc.dram_tensor("outT", [D, cfg.WPAD], bf16, kind="ExternalOutput")

    GW, TPG, CAP, CHUNK = cfg.GW, cfg.TPG, cfg.CAP, cfg.CHUNK
    NT = cfg.NWIN * 2                        # tiles per class

    with tile.TileContext(nc) as tc:
        with (
            tc.tile_pool(name="const", bufs=1) as cst,
            tc.tile_pool(name="spacc", bufs=1) as spa,
            tc.tile_pool(name="gath", bufs=18) as gp,
            tc.tile_pool(name="notq", bufs=18) as qp,
            tc.tile_pool(name="wio", bufs=4) as wio,
            tc.tile_pool(name="wrk", bufs=8) as wrk,
            tc.tile_pool(name="psw", bufs=3, space="PSUM") as pswp,
            tc.tile_pool(name="psd", bufs=1, space="PSUM") as psdp,
            tc.tile_pool(name="psc", bufs=2, space="PSUM") as pscp,
            tc.tile_pool(name="pst", bufs=1, space="PSUM") as pstp,
            tc.tile_pool(name="pso", bufs=1, space="PSUM") as psop,
        ):
            # ---- resident inputs
            gidx_t = cst.tile([128, cfg.TOK // 16], i16)
            wtgt_t = cst.tile([128, cfg.NCLS * NT], i16)
            sgidx_t = cst.tile([128, cfg.STOK // 16], i16)
            ssidx_t = cst.tile([128, cfg.STOK // 16], i16)
            iotar_t = cst.tile([128, 128 * TPG], i16)
            ident_t = cst.tile([128, 128], bf16)
            onesb_t = cst.tile([128, 1], bf16)
            negones_t = cst.tile([128, 1], bf16)
            onesrow_t = cst.tile([1, 128], bf16)
            onesrowf_t = cst.tile([1, 128], f32)
            wnT_t = cst.tile([D, D], bf16)
            wsT_t = cst.tile([D, D], bf16)
            bias_t = cst.tile([D, 1], f32)
            for t, d in ((gidx_t, gidx_d), (wtgt_t, wtgt_d),
                         (sgidx_t, sgidx_d), (ssidx_t, ssidx_d),
                         (iotar_t, iotar_d), (ident_t, ident_d),
                         (onesb_t, onesb_d), (negones_t, negones_d),
                         (onesrow_t, onesrow_d), (onesrowf_t, onesrowf_d),
                         (wnT_t, wnT_d),
                         (wsT_t, wsT_d), (bias_t, bias_d)):
                nc.sync.dma_start(t[:], d[:])

            # ---- spill: CCE scatter-add of negated features + degree
            own_t = spa.tile([P, (cfg.NWIN // 2) * cfg.SEW], bf16)
            peer_t = spa.tile([P, (cfg.NWIN // 2) * cfg.SEW], bf16)
            nc.vector.memset(own_t[:], 0.0)
            nc.vector.memset(peer_t[:], 0.0)
            SMS = cfg.SM // P
            spio_cm = tc.tile_pool(name="spio", bufs=3)
            spio = spio_cm.__enter__()
            sp_ts = []
            for ch in range(cfg.NSCH):
                bank = ch // cfg.NSUB
                sl = slice(ch * cfg.SM // 16, (ch + 1) * cfg.SM // 16)
                sp_t = spio.tile([P, SMS * D], f32, name="sp")
                nc.gpsimd.dma_gather(
                    out_ap=sp_t[:].rearrange("p (m e) -> p m e", e=D),
                    in_ap=x_d[bank * cfg.BANK:(bank + 1) * cfg.BANK, :],
                    idxs_ap=sgidx_t[:, sl],
                    num_idxs=cfg.SM,
                    num_idxs_reg=cfg.SM,
                    elem_size=D,
                    single_packet=False,
                )
                sp_ts.append(sp_t)
            aug_ts = []
            for ch in range(cfg.NSCH):
                aug_t = spio.tile([P, SMS * cfg.SEW], bf16, name="aug")
                aug3 = aug_t[:].rearrange("p (m e) -> p m e", e=cfg.SEW)
                nc.vector.memset(aug3[:, :, D:cfg.SEW], -1.0)
                nc.vector.tensor_scalar_mul(
                    aug3[:, :, 0:D],
                    sp_ts[ch][:].rearrange("p (m e) -> p m e", e=D), -1.0)
                aug_ts.append(aug3)
            for ch in range(cfg.NSCH):
                sl = slice(ch * cfg.SM // 16, (ch + 1) * cfg.SM // 16)
                nc.gpsimd.dma_scatter_add(
                    out_ap=own_t[:],
                    in_ap=aug_ts[ch],
                    idxs_ap=ssidx_t[:, sl],
                    num_idxs=cfg.SM,
                    num_idxs_reg=cfg.SM,
                    elem_size=cfg.SEW,
                    out_ap_other=peer_t[:],
                    parity_reg=0,
                    sbuf_tokens_per_rank=128,
                    single_packet=False,
                )

            spio_cm.__exit__(None, None, None)

            # ---- main: software-pipelined gather/notq (g) + windows (g-1)
            def emit_group_io(wbase, gw):
                tpg = gw * 2
                chunk = gw * CAP
                gts = []
                for k in range(cfg.NCLS):
                    bank = k // 2
                    base = k * cfg.SC + wbase * CAP
                    gt = gp.tile([P, TPG * 2 * D], bf16, name="gt")
                    nc.gpsimd.dma_gather(
                        out_ap=gt[:, :tpg * 2 * D].rearrange(
                            "p (m e) -> p m e", e=2 * D),
                        in_ap=xpair_d[bank * (cfg.BANK // 2):
                                      (bank + 1) * (cfg.BANK // 2), :],
                        idxs_ap=gidx_t[:, base // 16:(base + chunk) // 16],
                        num_idxs=chunk,
                        num_idxs_reg=chunk,
                        elem_size=2 * D,
                        single_packet=False,
                    )
                    gts.append(gt)
                qts = []
                for k in range(cfg.NCLS):
                    qt = qp.tile([P, 128 * TPG], bf16, name="qt")
                    wsl = wtgt_t[:, k * NT + wbase * 2:
                                 k * NT + wbase * 2 + tpg]
                    nc.vector.tensor_tensor(
                        qt[:, :128 * tpg].rearrange("p (j t) -> p j t",
                                                    t=tpg),
                        wsl.unsqueeze(1).broadcast_to([128, 128, tpg]),
                        iotar_t[:].rearrange("p (j t) -> p j t",
                                             t=TPG)[:, :, 0:tpg],
                        NE)
                    qts.append(qt)
                xTg_t = wio.tile([D, GW * P], bf16, name="xTg")
                nc.sync.dma_start(xTg_t[:, :gw * P],
                                  xT_d[:, wbase * P:(wbase + gw) * P])
                return gts, qts, xTg_t

            def emit_group_windows(wbase, gw, gts, qts, xTg_t):
                tpg = gw * 2
                outg_t = wio.tile([D, GW * P], bf16, name="outg")
                for wl in range(gw):
                    w = wbase + wl
                    psw = pswp.tile([P, D], f32, name="psw")
                    psd = psdp.tile([P, 1], f32, name="psd")
                    psc = pscp.tile([1, D], f32, name="psc")
                    nmm = cfg.NCLS * 2
                    i = 0
                    for k in range(cfg.NCLS):
                        side = k & 1
                        g3 = gts[k][:, :tpg * 2 * D].rearrange(
                            "p (m e) -> p m e", e=2 * D)
                        q3 = qts[k][:, :128 * tpg].rearrange(
                            "p (j t) -> p j t", t=tpg)
                        for h in range(2):
                            c = wl * 2 + h
                            lhsT = q3[:, :, c]
                            rhs = g3[:, c, side * D:(side + 1) * D]
                            st = i == 0
                            sp = i == nmm - 1
                            nc.tensor.matmul(psw[:], lhsT=lhsT, rhs=rhs,
                                             start=st, stop=False)
                            nc.tensor.matmul(psd[:], lhsT=lhsT,
                                             rhs=onesb_t[:],
                                             start=st, stop=sp)
                            nc.tensor.matmul(psc[:], lhsT=negones_t[:],
                                             rhs=rhs, start=st, stop=sp)
                            i += 1
                    # close psw: += onesrow^T @ psc  (broadcast -sum(feat))
                    corr_t = wrk.tile([1, D], f32, name="corr")
                    nc.scalar.copy(corr_t[:], psc[:])
                    nc.tensor.matmul(psw[:], lhsT=onesrowf_t[:],
                                     rhs=corr_t[:], start=False, stop=True)
                    # merge spill (negated partials): psw += spill_feat
                    sacc = own_t if (w & 1) == 0 else peer_t
                    so = (w >> 1) * cfg.SEW
                    nc.vector.tensor_tensor(psw[:], psw[:],
                                            sacc[:, so:so + D],
                                            mybir.AluOpType.add)
                    # -deg_total = (psd - 2048) + (-sdeg); clamp to <= -1
                    ndeg_t = wrk.tile([P, 1], f32, name="nd")
                    nc.vector.scalar_tensor_tensor(
                        ndeg_t[:], psd[:], -2048.0,
                        sacc[:, so + D:so + D + 1],
                        mybir.AluOpType.add, mybir.AluOpType.add)
                    ndegc_t = wrk.tile([P, 1], f32, name="ndc")
                    nc.vector.tensor_scalar_min(ndegc_t[:], ndeg_t[:], -1.0)
                    nrecip_t = wrk.tile([P, 1], f32, name="nrecip")
                    nc.vector.reciprocal(nrecip_t[:], ndegc_t[:])
                    mean_t = wrk.tile([P, D], bf16, name="mean")
                    nc.vector.tensor_scalar_mul(mean_t[:], psw[:],
                                                nrecip_t[:, 0:1])
                    psT = pstp.tile([D, P], bf16, name="psT")
                    nc.tensor.transpose(psT[:], mean_t[:], ident_t[:])
                    meanT_t = wrk.tile([D, P], bf16, name="meanT")
                    nc.scalar.copy(meanT_t[:], psT[:])
                    po = psop.tile([D, P], f32, name="po")
                    nc.tensor.matmul(po[:], lhsT=wnT_t[:], rhs=meanT_t[:],
                                     start=True, stop=False)
                    nc.tensor.matmul(po[:], lhsT=wsT_t[:],
                                     rhs=xTg_t[:, wl * P:(wl + 1) * P],
                                     start=False, stop=True)
                    nc.scalar.activation(outg_t[:, wl * P:(wl + 1) * P],
                                         po[:],
                                         mybir.ActivationFunctionType.Identity,
                                         bias=bias_t[:, 0:1])
                nc.sync.dma_start(outT_d[:, wbase * P:(wbase + gw) * P],
                                  outg_t[:, :gw * P])

            groups = [GW] * 12 + [5, 4, 3, 2]
            assert sum(groups) == cfg.NWIN
            pend = None
            wbase = 0
            for gi in range(len(groups) + 1):
                if gi < len(groups):
                    gw = groups[gi]
                    gts, qts, xTg_t = emit_group_io(wbase, gw)
                if pend is not None:
                    emit_group_windows(*pend)
                if gi < len(groups):
                    pend = (wbase, gw, gts, qts, xTg_t)
                    wbase += gw

    nc.compile()
    return nc


def _prep_inputs(cfg, x, edge_index, W_self, W_neigh, bias):
    """Host-side index bookkeeping: bucket edges by (window, class) with
    fixed 256-slot caps, route overflow to dup-free spill chunks, int16
    encode, 16-wrap gather indices."""
    import ml_dtypes

    bf16 = ml_dtypes.bfloat16
    x = np.ascontiguousarray(np.asarray(x, dtype=np.float32))
    ei = np.asarray(edge_index)
    row = ei[0].astype(np.int64)
    col = ei[1].astype(np.int64)

    xpair = np.ascontiguousarray(
        x.astype(bf16).reshape(cfg.N // 2, 2 * cfg.D))
    wnT = np.ascontiguousarray(np.asarray(W_neigh, np.float32).T.astype(bf16))
    wsT = np.ascontiguousarray(np.asarray(W_self, np.float32).T.astype(bf16))
    bias_c = np.ascontiguousarray(
        np.asarray(bias, np.float32).reshape(cfg.D, 1))
    iotar = np.ascontiguousarray(np.tile(
        np.repeat(np.arange(128, dtype=np.int16), cfg.TPG), (128, 1)))
    ident = np.ascontiguousarray(np.eye(128, dtype=np.float32).astype(bf16))
    onesb = np.ones((128, 1), dtype=bf16)
    negones = np.full((128, 1), -1.0, dtype=bf16)
    onesrow = np.ones((1, 128), dtype=bf16)

    def wrap16(a):
        return np.tile(np.ascontiguousarray(a.reshape(-1, 16).T), (8, 1))

    owner = col // cfg.W
    in_maps = []
    for c in range(cfg.NCORE):
        msk = owner == c
        r = row[msk]
        tl = (col[msk] - c * cfg.W).astype(np.int64)
        w = tl >> 7
        j = tl & 127
        bank = r // cfg.BANK
        L = r % cfg.BANK
        side = L & 1
        g16 = L >> 1
        cls = bank * 2 + side
        bucket = cls * cfg.NWIN + w

        # occurrence index within (bucket, target)
        n = len(tl)
        o1 = np.lexsort((tl, bucket))
        bt = bucket[o1] * 16384 + tl[o1]
        first = np.r_[True, bt[1:] != bt[:-1]]
        runstart = np.maximum.accumulate(np.where(first, np.arange(n), 0))
        occ_s = np.arange(n) - runstart
        occ = np.empty(n, np.int64)
        occ[o1] = occ_s

        # rank within bucket by occ DESCENDING -> first CAP stay, rest spill.
        # Main buckets tolerate any per-target multiplicity (matmul segsum),
        # spill chunks do not (CCE RMW races) -- so spill the occ-0 tokens,
        # which are distinct targets by construction.
        o2 = np.lexsort((-occ, bucket))
        b_s = bucket[o2]
        first_b = np.r_[True, b_s[1:] != b_s[:-1]]
        runstart_b = np.maximum.accumulate(np.where(first_b, np.arange(n), 0))
        rank_s = np.arange(n) - runstart_b
        rank = np.empty(n, np.int64)
        rank[o2] = rank_s

        main = rank < cfg.CAP
        slot = cls[main] * cfg.SC + w[main] * cfg.CAP + rank[main]
        gbuf = np.zeros(cfg.TOK, np.int16)
        tbuf = np.full(cfg.TOK, cfg.PADTGT, np.int16)
        gbuf[slot] = g16[main].astype(np.int16)
        tbuf[slot] = j[main].astype(np.int16)

        # spill: per (bank, target) round-robin over NSUB chunks
        sm = ~main
        sb = bank[sm]
        sL = L[sm]
        stl = tl[sm]
        ns = len(stl)
        sgbuf = np.zeros(cfg.STOK, np.int16)
        ssbuf = np.full(cfg.STOK, cfg.DEAD, np.int16)
        if ns:
            o3 = np.lexsort((stl, sb))
            key = sb[o3] * 16384 + stl[o3]
            f3 = np.r_[True, key[1:] != key[:-1]]
            rs3 = np.maximum.accumulate(np.where(f3, np.arange(ns), 0))
            occ3_s = np.arange(ns) - rs3
            occ3 = np.empty(ns, np.int64)
            occ3[o3] = occ3_s
            if occ3.max() >= cfg.NSUB:
                raise RuntimeError(
                    f"spill multiplicity {occ3.max() + 1} > NSUB={cfg.NSUB}")
            sub = (occ3 + ((stl * 2654435761) >> 8)) % cfg.NSUB
            chunk = sb * cfg.NSUB + sub
            o4 = np.lexsort((np.arange(ns), chunk))
            ch_s = chunk[o4]
            f4 = np.r_[True, ch_s[1:] != ch_s[:-1]]
            rs4 = np.maximum.accumulate(np.where(f4, np.arange(ns), 0))
            pos_s = np.arange(ns) - rs4
            pos = np.empty(ns, np.int64)
            pos[o4] = pos_s
            if pos.max() >= cfg.SM:
                raise RuntimeError(
                    f"spill chunk overflow: {pos.max() + 1} > SM={cfg.SM}")
            sslot = chunk * cfg.SM + pos
            sgbuf[sslot] = sL.astype(np.int16)
            ssbuf[sslot] = stl.astype(np.int16)

        # wtgt: [128, NCLS * NWIN * 2], column = class-major tile index,
        # wtgt[p, t] = window-relative target of slot t*128 + p
        wtgt = np.ascontiguousarray(
            tbuf.reshape(cfg.NCLS * cfg.NWIN * 2, 128).T)

        xw = np.zeros((cfg.D, cfg.WPAD), np.float32)
        xw[:, :cfg.W] = x[c * cfg.W:(c + 1) * cfg.W].T
        in_maps.append({
            "xpair": xpair,
            "x": x,
            "xT": np.ascontiguousarray(xw.astype(bf16)),
            "gidx": wrap16(gbuf),
            "wtgt": wtgt,
            "sgidx": wrap16(sgbuf),
            "ssidx": wrap16(ssbuf),
            "iotar": iotar,
            "ident": ident,
            "onesb": onesb,
            "negones": negones,
            "onesrow": onesrow,
            "onesrowf": np.ones((1, 128), np.float32),
            "wnT": wnT,
            "wsT": wsT,
            "bias": bias_c,
        })
    return in_maps


_CACHED_NC = None


def _get_nc():
    global _CACHED_NC
    if _CACHED_NC is None:
        _CACHED_NC = _build(CFG)
    return _CACHED_NC


def kernel(x, edge_index, W_self, W_neigh, bias, _trace=False,
           _trace_kwargs=None):
    from concourse.bass_utils import run_bass_kernel_spmd

    cfg = CFG
    nc = _get_nc()
    in_maps = _prep_inputs(cfg, x, edge_index, W_self, W_neigh, bias)
    kw = {}
    if _trace:
        kw["trace"] = True
        if _trace_kwargs:
            kw.update(_trace_kwargs)
    res = run_bass_kernel_spmd(nc, in_maps, list(range(cfg.NCORE)), **kw)
    out = np.concatenate(
        [res.results[c]["outT"][:, :cfg.W].T for c in range(cfg.NCORE)],
        axis=0)
    out = np.ascontiguousarray(out, dtype=np.float32)
    if _trace:
        return out, res
    return out


# revision 8
# speedup vs baseline: 1.0706x; 1.0010x over previous
"""DimGraphConv (GNN mean-aggregation message passing) on 8 Trainium2 cores.

Math (per reference):
    out = x @ W_self.T + segment_mean(x[row], col) @ W_neigh.T + bias

v3 design: target-sharded (core c owns targets [c*12500, (c+1)*12500)); the
per-edge aggregation runs on the TENSOR engine instead of the CCE scatter-add:

  * Edges are bucketed by (128-target window w, gather class k).  A class is
    (source bank, row parity): x is stored as a bf16 row-pair array
    xpair[50000, 128] so one 256B gather descriptor fetches the row pair that
    contains the wanted source row; parity picks the 64-element half.
  * Each bucket has a fixed 256-slot capacity (2 matmul tiles).  Overflow
    (~2.5% of edges) is routed to a small CCE scatter-add spill path.
  * Per 128-token tile, DVE builds notq[tok, j, t] = (wtgt[tok,t] != j) in
    bf16 — j-major layout so all operands are stride-1-packed 2-byte (DVE 2x
    mode); iota is materialized as an uploaded iotarep const.  PE computes
        psw[:, 0:64] += notq_t^T @ feat_t     (= sum(feat) - agg  per target)
        psw[:, 64]   += notq_t^T @ ones       (= 128*T - deg      per target)
        psc          += (-ones)^T @ feat_t    (= -sum(feat), a [1,64] row)
    and a per-window broadcast matmul psw[:,0:64] += onesrow^T @ psc makes
    psw = -agg.  Spill partials (negated on device) merge in with a DVE add.
    mean = psw / min(psd - 2048 - sdeg, -1)  (negative / negative).
  * Window close: PE transpose of mean, then two bf16 GEMMs
    out^T = W_neigh@mean^T + W_self@x^T, ACT adds bias.
  * Software pipelining: window-processing of group g-1 is emitted after the
    gathers+notq of group g, so per-engine streams never head-of-line block
    on cross-engine dependencies.

All heavy DMA is the gather stream: ~200k 256B descriptors per core.
Host-side prep is pure index bookkeeping (bucketing / padding / int16
encoding); all feature math happens on device.
"""
import sys

sys.path.insert(0, "/opt/trn_rl_repo")

import numpy as np


# ---------------------------------------------------------------- config
class _Cfg:
    N = 100000          # nodes
    D = 64              # feature dim
    NCORE = 8
    W = 12500           # targets per core
    NWIN = 98           # 128-target windows per core (98*128 = 12544)
    WPAD = 12544
    NBANK = 4           # source banks of 25000 rows (int16 gather idx limit)
    BANK = 25000
    NCLS = 8            # gather classes: bank * 2 + row parity
    CAP = 256           # bucket capacity (slots per (window, class))
    GW = 7              # windows per gather group
    NG = 14             # groups (NG * GW == NWIN)
    DEAD = 12543        # dead spill cell (padded target, discarded)
    PADTGT = 1000       # wtgt value for pad slots (!= any window-rel target)
    # spill (CCE scatter-add) path
    NSUB = 2            # spill chunks per bank (dup-free round robin)
    SM = 896            # spill chunk capacity (7*128)
    SEW = 68            # spill payload elems: 64 feat + 1 deg + 3 pad

    @property
    def SC(self):
        return self.NWIN * self.CAP          # slots per class (25088)

    @property
    def TOK(self):
        return self.NCLS * self.SC           # total main slots (200704)

    @property
    def CHUNK(self):
        return self.GW * self.CAP            # tokens per (class, group) gather

    @property
    def TPG(self):
        return self.GW * 2                   # tiles per (class, group)

    @property
    def NSCH(self):
        return self.NBANK * self.NSUB        # spill chunks

    @property
    def STOK(self):
        return self.NSCH * self.SM


CFG = _Cfg()


def _build(cfg):
    """Build + compile the SPMD Bass program. Returns the Bacc instance."""
    import concourse.tile as tile
    from concourse import bacc, mybir

    P = 128
    D = cfg.D
    f32 = mybir.dt.float32
    bf16 = mybir.dt.bfloat16
    i16 = mybir.dt.int16
    NE = mybir.AluOpType.not_equal

    nc = bacc.Bacc(None, target_bir_lowering=False, debug=False)
    xpair_d = nc.dram_tensor("xpair", [cfg.N // 2, 2 * D], bf16,
                             kind="ExternalInput")
    x_d = nc.dram_tensor("x", [cfg.N, D], f32, kind="ExternalInput")
    xT_d = nc.dram_tensor("xT", [D, cfg.WPAD], bf16, kind="ExternalInput")
    gidx_d = nc.dram_tensor("gidx", [128, cfg.TOK // 16], i16,
                            kind="ExternalInput")
    wtgt_d = nc.dram_tensor("wtgt", [128, cfg.NCLS * cfg.NWIN * 2], i16,
                            kind="ExternalInput")
    sgidx_d = nc.dram_tensor("sgidx", [128, cfg.STOK // 16], i16,
                             kind="ExternalInput")
    ssidx_d = nc.dram_tensor("ssidx", [128, cfg.STOK // 16], i16,
                             kind="ExternalInput")
    iotar_d = nc.dram_tensor("iotar", [128, 128 * cfg.TPG], i16,
                             kind="ExternalInput")
    ident_d = nc.dram_tensor("ident", [128, 128], bf16, kind="ExternalInput")
    onesb_d = nc.dram_tensor("onesb", [128, 1], bf16, kind="ExternalInput")
    negones_d = nc.dram_tensor("negones", [128, 1], bf16,
                               kind="ExternalInput")
    onesrow_d = nc.dram_tensor("onesrow", [1, 128], bf16,
                               kind="ExternalInput")
    onesrowf_d = nc.dram_tensor("onesrowf", [1, 128], f32,
                                kind="ExternalInput")
    wnT_d = nc.dram_tensor("wnT", [D, D], bf16, kind="ExternalInput")
    wsT_d = nc.dram_tensor("wsT", [D, D], bf16, kind="ExternalInput")
    bias_d = nc.dram_tensor("bias", [D, 1], f32, kind="ExternalInput")
    outT_d = nc.dram_tensor("outT", [D, cfg.WPAD], bf16, kind="ExternalOutput")

    GW, TPG, CAP, CHUNK = cfg.GW, cfg.TPG, cfg.CAP, cfg.CHUNK
    NT = cfg.NWIN * 2                        # tiles per class

    with tile.TileContext(nc) as tc:
        with (
            tc.tile_pool(name="const", bufs=1) as cst,
            tc.tile_pool(name="spacc", bufs=1) as spa,
            tc.tile_pool(name="gath", bufs=18) as gp,
            tc.tile_pool(name="notq", bufs=18) as qp,
            tc.tile_pool(name="wio", bufs=4) as wio,
            tc.tile_pool(name="wrk", bufs=8) as wrk,
            tc.tile_pool(name="psw", bufs=3, space="PSUM") as pswp,
            tc.tile_pool(name="psd", bufs=1, space="PSUM") as psdp,
            tc.tile_pool(name="psc", bufs=2, space="PSUM") as pscp,
            tc.tile_pool(name="pst", bufs=1, space="PSUM") as pstp,
            tc.tile_pool(name="pso", bufs=1, space="PSUM") as psop,
        ):
            # ---- resident inputs
            gidx_t = cst.tile([128, cfg.TOK // 16], i16)
            wtgt_t = cst.tile([128, cfg.NCLS * NT], i16)
            sgidx_t = cst.tile([128, cfg.STOK // 16], i16)
            ssidx_t = cst.tile([128, cfg.STOK // 16], i16)
            iotar_t = cst.tile([128, 128 * TPG], i16)
            ident_t = cst.tile([128, 128], bf16)
            onesb_t = cst.tile([128, 1], bf16)
            negones_t = cst.tile([128, 1], bf16)
            onesrow_t = cst.tile([1, 128], bf16)
            onesrowf_t = cst.tile([1, 128], f32)
            wnT_t = cst.tile([D, D], bf16)
            wsT_t = cst.tile([D, D], bf16)
            bias_t = cst.tile([D, 1], f32)
            for t, d in ((gidx_t, gidx_d), (wtgt_t, wtgt_d),
                         (sgidx_t, sgidx_d), (ssidx_t, ssidx_d),
                         (iotar_t, iotar_d), (ident_t, ident_d),
                         (onesb_t, onesb_d), (negones_t, negones_d),
                         (onesrow_t, onesrow_d), (onesrowf_t, onesrowf_d),
                         (wnT_t, wnT_d),
                         (wsT_t, wsT_d), (bias_t, bias_d)):
                nc.sync.dma_start(t[:], d[:])

            # ---- spill: CCE scatter-add of negated features + degree
            own_t = spa.tile([P, (cfg.NWIN // 2) * cfg.SEW], bf16)
            peer_t = spa.tile([P, (cfg.NWIN // 2) * cfg.SEW], bf16)
            nc.vector.memset(own_t[:], 0.0)
            nc.vector.memset(peer_t[:], 0.0)
            SMS = cfg.SM // P
            spaug_cm = tc.tile_pool(name="spaug", bufs=8)
            spaug = spaug_cm.__enter__()
            spio_cm = tc.tile_pool(name="spio", bufs=3)
            spio = spio_cm.__enter__()
            sp_ts = []
            for ch in range(cfg.NSCH):
                bank = ch // cfg.NSUB
                sl = slice(ch * cfg.SM // 16, (ch + 1) * cfg.SM // 16)
                sp_t = spio.tile([P, SMS * D], f32, name="sp")
                nc.gpsimd.dma_gather(
                    out_ap=sp_t[:].rearrange("p (m e) -> p m e", e=D),
                    in_ap=x_d[bank * cfg.BANK:(bank + 1) * cfg.BANK, :],
                    idxs_ap=sgidx_t[:, sl],
                    num_idxs=cfg.SM,
                    num_idxs_reg=cfg.SM,
                    elem_size=D,
                    single_packet=False,
                )
                sp_ts.append(sp_t)
            aug_ts = []
            for ch in range(cfg.NSCH):
                aug_t = spaug.tile([P, SMS * cfg.SEW], bf16, name="aug")
                aug3 = aug_t[:].rearrange("p (m e) -> p m e", e=cfg.SEW)
                nc.vector.memset(aug3[:, :, D:cfg.SEW], -1.0)
                nc.vector.tensor_scalar_mul(
                    aug3[:, :, 0:D],
                    sp_ts[ch][:].rearrange("p (m e) -> p m e", e=D), -1.0)
                aug_ts.append(aug3)
            spio_cm.__exit__(None, None, None)

            sc_next = [0]

            def emit_one_scatter():
                ch = sc_next[0]
                if ch >= cfg.NSCH:
                    return
                sc_next[0] += 1
                sl = slice(ch * cfg.SM // 16, (ch + 1) * cfg.SM // 16)
                nc.gpsimd.dma_scatter_add(
                    out_ap=own_t[:],
                    in_ap=aug_ts[ch],
                    idxs_ap=ssidx_t[:, sl],
                    num_idxs=cfg.SM,
                    num_idxs_reg=cfg.SM,
                    elem_size=cfg.SEW,
                    out_ap_other=peer_t[:],
                    parity_reg=0,
                    sbuf_tokens_per_rank=128,
                    single_packet=False,
                )
                if sc_next[0] == cfg.NSCH:
                    spaug_cm.__exit__(None, None, None)

            # ---- main: software-pipelined gather/notq (g) + windows (g-1)
            def emit_group_io(wbase, gw):
                tpg = gw * 2
                chunk = gw * CAP
                gts = []
                for k in range(cfg.NCLS):
                    bank = k // 2
                    base = k * cfg.SC + wbase * CAP
                    gt = gp.tile([P, TPG * 2 * D], bf16, name="gt")
                    nc.gpsimd.dma_gather(
                        out_ap=gt[:, :tpg * 2 * D].rearrange(
                            "p (m e) -> p m e", e=2 * D),
                        in_ap=xpair_d[bank * (cfg.BANK // 2):
                                      (bank + 1) * (cfg.BANK // 2), :],
                        idxs_ap=gidx_t[:, base // 16:(base + chunk) // 16],
                        num_idxs=chunk,
                        num_idxs_reg=chunk,
                        elem_size=2 * D,
                        single_packet=False,
                    )
                    gts.append(gt)
                    if k % 2 == 0:
                        emit_one_scatter()
                qts = []
                for k in range(cfg.NCLS):
                    qt = qp.tile([P, 128 * TPG], bf16, name="qt")
                    wsl = wtgt_t[:, k * NT + wbase * 2:
                                 k * NT + wbase * 2 + tpg]
                    nc.vector.tensor_tensor(
                        qt[:, :128 * tpg].rearrange("p (j t) -> p j t",
                                                    t=tpg),
                        wsl.unsqueeze(1).broadcast_to([128, 128, tpg]),
                        iotar_t[:].rearrange("p (j t) -> p j t",
                                             t=TPG)[:, :, 0:tpg],
                        NE)
                    qts.append(qt)
                xTg_t = wio.tile([D, GW * P], bf16, name="xTg")
                nc.sync.dma_start(xTg_t[:, :gw * P],
                                  xT_d[:, wbase * P:(wbase + gw) * P])
                return gts, qts, xTg_t

            def emit_group_windows(wbase, gw, gts, qts, xTg_t):
                tpg = gw * 2
                outg_t = wio.tile([D, GW * P], bf16, name="outg")
                for wl in range(gw):
                    w = wbase + wl
                    psw = pswp.tile([P, D], f32, name="psw")
                    psd = psdp.tile([P, 1], f32, name="psd")
                    psc = pscp.tile([1, D], f32, name="psc")
                    nmm = cfg.NCLS * 2
                    i = 0
                    for k in range(cfg.NCLS):
                        side = k & 1
                        g3 = gts[k][:, :tpg * 2 * D].rearrange(
                            "p (m e) -> p m e", e=2 * D)
                        q3 = qts[k][:, :128 * tpg].rearrange(
                            "p (j t) -> p j t", t=tpg)
                        for h in range(2):
                            c = wl * 2 + h
                            lhsT = q3[:, :, c]
                            rhs = g3[:, c, side * D:(side + 1) * D]
                            st = i == 0
                            sp = i == nmm - 1
                            nc.tensor.matmul(psw[:], lhsT=lhsT, rhs=rhs,
                                             start=st, stop=False)
                            nc.tensor.matmul(psd[:], lhsT=lhsT,
                                             rhs=onesb_t[:],
                                             start=st, stop=sp)
                            nc.tensor.matmul(psc[:], lhsT=negones_t[:],
                                             rhs=rhs, start=st, stop=sp)
                            i += 1
                    # close psw: += onesrow^T @ psc  (broadcast -sum(feat))
                    corr_t = wrk.tile([1, D], f32, name="corr")
                    nc.scalar.copy(corr_t[:], psc[:])
                    nc.tensor.matmul(psw[:], lhsT=onesrowf_t[:],
                                     rhs=corr_t[:], start=False, stop=True)
                    # merge spill (negated partials): psw += spill_feat
                    sacc = own_t if (w & 1) == 0 else peer_t
                    so = (w >> 1) * cfg.SEW
                    nc.vector.tensor_tensor(psw[:], psw[:],
                                            sacc[:, so:so + D],
                                            mybir.AluOpType.add)
                    # -deg_total = (psd - 2048) + (-sdeg); clamp to <= -1
                    ndeg_t = wrk.tile([P, 1], f32, name="nd")
                    nc.vector.scalar_tensor_tensor(
                        ndeg_t[:], psd[:], -2048.0,
                        sacc[:, so + D:so + D + 1],
                        mybir.AluOpType.add, mybir.AluOpType.add)
                    ndegc_t = wrk.tile([P, 1], f32, name="ndc")
                    nc.vector.tensor_scalar_min(ndegc_t[:], ndeg_t[:], -1.0)
                    nrecip_t = wrk.tile([P, 1], f32, name="nrecip")
                    nc.vector.reciprocal(nrecip_t[:], ndegc_t[:])
                    mean_t = wrk.tile([P, D], bf16, name="mean")
                    nc.vector.tensor_scalar_mul(mean_t[:], psw[:],
                                                nrecip_t[:, 0:1])
                    psT = pstp.tile([D, P], bf16, name="psT")
                    nc.tensor.transpose(psT[:], mean_t[:], ident_t[:])
                    meanT_t = wrk.tile([D, P], bf16, name="meanT")
                    nc.scalar.copy(meanT_t[:], psT[:])
                    po = psop.tile([D, P], f32, name="po")
                    nc.tensor.matmul(po[:], lhsT=wnT_t[:], rhs=meanT_t[:],
                                     start=True, stop=False)
                    nc.tensor.matmul(po[:], lhsT=wsT_t[:],
                                     rhs=xTg_t[:, wl * P:(wl + 1) * P],
                                     start=False, stop=True)
                    nc.scalar.activation(outg_t[:, wl * P:(wl + 1) * P],
                                         po[:],
                                         mybir.ActivationFunctionType.Identity,
                                         bias=bias_t[:, 0:1])
                    if wbase >= 89:
                        nc.sync.dma_start(outT_d[:, w * P:(w + 1) * P],
                                          outg_t[:, wl * P:(wl + 1) * P])
                if wbase < 89:
                    nc.sync.dma_start(outT_d[:, wbase * P:(wbase + gw) * P],
                                      outg_t[:, :gw * P])

            groups = [GW] * 12 + [5, 4, 3, 2]
            assert sum(groups) == cfg.NWIN
            pend = None
            wbase = 0
            for gi in range(len(groups) + 1):
                if gi < len(groups):
                    gw = groups[gi]
                    gts, qts, xTg_t = emit_group_io(wbase, gw)
                if pend is not None:
                    emit_group_windows(*pend)
                if gi < len(groups):
                    pend = (wbase, gw, gts, qts, xTg_t)
                    wbase += gw

    nc.compile()
    return nc


def _prep_inputs(cfg, x, edge_index, W_self, W_neigh, bias):
    """Host-side index bookkeeping: bucket edges by (window, class) with
    fixed 256-slot caps, route overflow to dup-free spill chunks, int16
    encode, 16-wrap gather indices."""
    import ml_dtypes

    bf16 = ml_dtypes.bfloat16
    x = np.ascontiguousarray(np.asarray(x, dtype=np.float32))
    ei = np.asarray(edge_index)
    row = ei[0].astype(np.int64)
    col = ei[1].astype(np.int64)

    xpair = np.ascontiguousarray(
        x.astype(bf16).reshape(cfg.N // 2, 2 * cfg.D))
    wnT = np.ascontiguousarray(np.asarray(W_neigh, np.float32).T.astype(bf16))
    wsT = np.ascontiguousarray(np.asarray(W_self, np.float32).T.astype(bf16))
    bias_c = np.ascontiguousarray(
        np.asarray(bias, np.float32).reshape(cfg.D, 1))
    iotar = np.ascontiguousarray(np.tile(
        np.repeat(np.arange(128, dtype=np.int16), cfg.TPG), (128, 1)))
    ident = np.ascontiguousarray(np.eye(128, dtype=np.float32).astype(bf16))
    onesb = np.ones((128, 1), dtype=bf16)
    negones = np.full((128, 1), -1.0, dtype=bf16)
    onesrow = np.ones((1, 128), dtype=bf16)

    def wrap16(a):
        return np.tile(np.ascontiguousarray(a.reshape(-1, 16).T), (8, 1))

    owner = col // cfg.W
    in_maps = []
    for c in range(cfg.NCORE):
        msk = owner == c
        r = row[msk]
        tl = (col[msk] - c * cfg.W).astype(np.int64)
        w = tl >> 7
        j = tl & 127
        bank = r // cfg.BANK
        L = r % cfg.BANK
        side = L & 1
        g16 = L >> 1
        cls = bank * 2 + side
        bucket = cls * cfg.NWIN + w

        # occurrence index within (bucket, target)
        n = len(tl)
        o1 = np.lexsort((tl, bucket))
        bt = bucket[o1] * 16384 + tl[o1]
        first = np.r_[True, bt[1:] != bt[:-1]]
        runstart = np.maximum.accumulate(np.where(first, np.arange(n), 0))
        occ_s = np.arange(n) - runstart
        occ = np.empty(n, np.int64)
        occ[o1] = occ_s

        # rank within bucket by occ DESCENDING -> first CAP stay, rest spill.
        # Main buckets tolerate any per-target multiplicity (matmul segsum),
        # spill chunks do not (CCE RMW races) -- so spill the occ-0 tokens,
        # which are distinct targets by construction.
        o2 = np.lexsort((-occ, bucket))
        b_s = bucket[o2]
        first_b = np.r_[True, b_s[1:] != b_s[:-1]]
        runstart_b = np.maximum.accumulate(np.where(first_b, np.arange(n), 0))
        rank_s = np.arange(n) - runstart_b
        rank = np.empty(n, np.int64)
        rank[o2] = rank_s

        main = rank < cfg.CAP
        slot = cls[main] * cfg.SC + w[main] * cfg.CAP + rank[main]
        gbuf = np.zeros(cfg.TOK, np.int16)
        tbuf = np.full(cfg.TOK, cfg.PADTGT, np.int16)
        gbuf[slot] = g16[main].astype(np.int16)
        tbuf[slot] = j[main].astype(np.int16)

        # spill: per (bank, target) round-robin over NSUB chunks
        sm = ~main
        sb = bank[sm]
        sL = L[sm]
        stl = tl[sm]
        ns = len(stl)
        sgbuf = np.zeros(cfg.STOK, np.int16)
        ssbuf = np.full(cfg.STOK, cfg.DEAD, np.int16)
        if ns:
            o3 = np.lexsort((stl, sb))
            key = sb[o3] * 16384 + stl[o3]
            f3 = np.r_[True, key[1:] != key[:-1]]
            rs3 = np.maximum.accumulate(np.where(f3, np.arange(ns), 0))
            occ3_s = np.arange(ns) - rs3
            occ3 = np.empty(ns, np.int64)
            occ3[o3] = occ3_s
            if occ3.max() >= cfg.NSUB:
                raise RuntimeError(
                    f"spill multiplicity {occ3.max() + 1} > NSUB={cfg.NSUB}")
            sub = (occ3 + ((stl * 2654435761) >> 8)) % cfg.NSUB
            chunk = sb * cfg.NSUB + sub
            o4 = np.lexsort((np.arange(ns), chunk))
            ch_s = chunk[o4]
            f4 = np.r_[True, ch_s[1:] != ch_s[:-1]]
            rs4 = np.maximum.accumulate(np.where(f4, np.arange(ns), 0))
            pos_s = np.arange(ns) - rs4
            pos = np.empty(ns, np.int64)
            pos[o4] = pos_s
            if pos.max() >= cfg.SM:
                raise RuntimeError(
                    f"spill chunk overflow: {pos.max() + 1} > SM={cfg.SM}")
            sslot = chunk * cfg.SM + pos
            sgbuf[sslot] = sL.astype(np.int16)
            ssbuf[sslot] = stl.astype(np.int16)

        # wtgt: [128, NCLS * NWIN * 2], column = class-major tile index,
        # wtgt[p, t] = window-relative target of slot t*128 + p
        wtgt = np.ascontiguousarray(
            tbuf.reshape(cfg.NCLS * cfg.NWIN * 2, 128).T)

        xw = np.zeros((cfg.D, cfg.WPAD), np.float32)
        xw[:, :cfg.W] = x[c * cfg.W:(c + 1) * cfg.W].T
        in_maps.append({
            "xpair": xpair,
            "x": x,
            "xT": np.ascontiguousarray(xw.astype(bf16)),
            "gidx": wrap16(gbuf),
            "wtgt": wtgt,
            "sgidx": wrap16(sgbuf),
            "ssidx": wrap16(ssbuf),
            "iotar": iotar,
            "ident": ident,
            "onesb": onesb,
            "negones": negones,
            "onesrow": onesrow,
            "onesrowf": np.ones((1, 128), np.float32),
            "wnT": wnT,
            "wsT": wsT,
            "bias": bias_c,
        })
    return in_maps


_CACHED_NC = None


def _get_nc():
    global _CACHED_NC
    if _CACHED_NC is None:
        _CACHED_NC = _build(CFG)
    return _CACHED_NC


def kernel(x, edge_index, W_self, W_neigh, bias, _trace=False,
           _trace_kwargs=None):
    from concourse.bass_utils import run_bass_kernel_spmd

    cfg = CFG
    nc = _get_nc()
    in_maps = _prep_inputs(cfg, x, edge_index, W_self, W_neigh, bias)
    kw = {}
    if _trace:
        kw["trace"] = True
        if _trace_kwargs:
            kw.update(_trace_kwargs)
    res = run_bass_kernel_spmd(nc, in_maps, list(range(cfg.NCORE)), **kw)
    out = np.concatenate(
        [res.results[c]["outT"][:, :cfg.W].T for c in range(cfg.NCORE)],
        axis=0)
    out = np.ascontiguousarray(out, dtype=np.float32)
    if _trace:
        return out, res
    return out
